# revision 14
# baseline (speedup 1.0000x reference)
"""LocationAwareAttention Trainium2 kernel.

Full inputs in, full outputs out. Internally shards the batch dim (B=32)
across 8 NeuronCores (4 batches each) and runs a Bass/Tile kernel per core.

Math (per batch b):
    conv_feat = Conv1d(prev_attn, k=3, pad=1)           (V, C)
    e  = tanh(qp + value@w_v.T + conv_feat@w_loc.T + bias)   (V, DIM)
    score = e @ w_score + b_score                        (V,)
    s = sigmoid(score);  attn = s / sum(s)               (V,)
    context = attn @ value                               (H,)
    out = concat(context, query) @ w_out.T + b_out       (H,)

Device-side restructuring:
  - e computed transposed [DIM, V]: psum accumulation of
      (a) w_vT-tiles.T @ value_t-tiles     (K=h, 8 tiles)
      (b) W2.T @ shifted-prev-attn         (K=3; conv folded into W2)
    then ACT tanh with per-partition bias qpb = qp + bias + w_loc@conv_b.
  - score via M=1 matmuls (lhsT = w_score column per d-group).
  - s transposed to columns via K=1 matmuls against a ones scalar.
  - context accumulated unnormalized (U = sum_v s_v * value_v) during the
    same single pass over value (natural layout), scaled by 1/S at the end.
  - out_proj batched over the 4 local batches with M=4 (combined^T columns).
"""

import ml_dtypes
import numpy as np

B, VLEN, H, DIM, C = 32, 2048, 1024, 1024, 32
NCORES = 8
BPC = B // NCORES          # batches per core
CHUNK = 512                # v-chunk
NCHUNK = VLEN // CHUNK     # 4
VT_PER_CHUNK = CHUNK // 128  # 4 v-tiles of 128 per chunk
NDG = DIM // 128           # 8 d-groups
NHT = H // 128             # 8 h-tiles
NKT = 2 * H // 128         # 16 k-tiles for out_proj

_CACHE = {}


def _build():
    from contextlib import ExitStack

    import concourse.bacc as bacc
    import concourse.mybir as mybir
    import concourse.tile as tile

    F32R = mybir.dt.float32r
    F32 = mybir.dt.float32
    BF16 = mybir.dt.bfloat16
    AF = mybir.ActivationFunctionType

    nc = bacc.Bacc("TRN2", target_bir_lowering=False)

    value_t = nc.declare_dram_parameter("value_t", [BPC, H, VLEN], BF16, isOutput=False)
    value_n = nc.declare_dram_parameter("value_n", [BPC, VLEN, H], BF16, isOutput=False)
    w_vT = nc.declare_dram_parameter("w_vT", [H, DIM], BF16, isOutput=False)
    w2 = nc.declare_dram_parameter("w2", [3, DIM], BF16, isOutput=False)
    pa_sh = nc.declare_dram_parameter("pa_sh", [BPC, 3, VLEN], BF16, isOutput=False)
    qpb = nc.declare_dram_parameter("qpb", [128, BPC * NDG], F32, isOutput=False)
    w_sc = nc.declare_dram_parameter("w_sc", [128, NDG], BF16, isOutput=False)
    b_sc = nc.declare_dram_parameter("b_sc", [1, 1], F32, isOutput=False)
    qT = nc.declare_dram_parameter("qT", [128, NHT * BPC], BF16, isOutput=False)
    w_outT = nc.declare_dram_parameter("w_outT", [2 * H, H], BF16, isOutput=False)
    b_outr = nc.declare_dram_parameter("b_outr", [BPC, H], F32, isOutput=False)
    attn_o = nc.declare_dram_parameter("attn_o", [BPC, VLEN], F32, isOutput=True)
    out_o = nc.declare_dram_parameter("out_o", [BPC, H], F32, isOutput=True)

    with tile.TileContext(nc) as tc, ExitStack() as ctx:
        cpool = ctx.enter_context(tc.tile_pool(name="const", bufs=1))
        vt_pool = ctx.enter_context(tc.tile_pool(name="vt", bufs=3))
        vn_pool = ctx.enter_context(tc.tile_pool(name="vn", bufs=3))
        t_pool = ctx.enter_context(tc.tile_pool(name="t", bufs=9))
        srow_pool = ctx.enter_context(tc.tile_pool(name="srow", bufs=2))
        scol_pool = ctx.enter_context(tc.tile_pool(name="scol", bufs=2))
        misc_pool = ctx.enter_context(tc.tile_pool(name="misc", bufs=2))
        pa_pool = ctx.enter_context(tc.tile_pool(name="pa", bufs=2))
        wo_pool = ctx.enter_context(tc.tile_pool(name="wo", bufs=6))

        eps_pool = ctx.enter_context(tc.tile_pool(name="eps", bufs=2, space="PSUM"))
        sc_pool = ctx.enter_context(tc.tile_pool(name="scps", bufs=2, space="PSUM"))
        u_pool = ctx.enter_context(tc.tile_pool(name="ups", bufs=2, space="PSUM"))

        # ---- constants / weights (loaded once) ----
        wv_sb = cpool.tile([128, NHT * DIM], BF16, tag="wv")
        nc.sync.dma_start(
            wv_sb[:].rearrange("p (ht d) -> p ht d", d=DIM),
            w_vT.rearrange("(ht p) d -> p ht d", p=128),
        )
        w2_sb = cpool.tile([3, DIM], BF16, tag="w2")
        nc.sync.dma_start(w2_sb[:], w2[:])
        qpb_sb = cpool.tile([128, BPC * NDG], F32, tag="qpb")
        nc.sync.dma_start(qpb_sb[:], qpb[:])
        wsc_sb = cpool.tile([128, NDG], BF16, tag="wsc")
        nc.sync.dma_start(wsc_sb[:], w_sc[:])
        bsc_sb = cpool.tile([1, 1], F32, tag="bsc")
        nc.sync.dma_start(bsc_sb[:], b_sc[:])
        bo_sb = cpool.tile([BPC, H], F32, tag="bo")
        nc.sync.dma_start(bo_sb[:], b_outr[:])
        one32 = cpool.tile([1, 1], F32, tag="one32")
        nc.vector.memset(one32[:], 1.0)

        # combined^T columns [k-part, kt*BPC + b]; query half DMA'd directly
        comb_sb = cpool.tile([128, NKT * BPC], BF16, tag="comb")
        nc.sync.dma_start(comb_sb[:, NHT * BPC : NKT * BPC], qT[:])

        out_sb = cpool.tile([BPC, H], F32, tag="outsb")

        for b in range(BPC):
            s_row = srow_pool.tile([1, VLEN], F32)
            sparts = misc_pool.tile([1, NCHUNK], F32, tag="sparts")
            s_col = scol_pool.tile([128, VLEN // 128], BF16)
            u_ps = u_pool.tile([1, H], F32)
            ssum = misc_pool.tile([1, 1], F32, tag="ssum")

            for ck in range(NCHUNK):
                vt_sb = vt_pool.tile([128, NHT * CHUNK], BF16)
                nc.sync.dma_start(
                    vt_sb[:].rearrange("p (ht v) -> p ht v", v=CHUNK),
                    value_t[b].rearrange("(ht p) v -> p ht v", p=128)[
                        :, :, ck * CHUNK : (ck + 1) * CHUNK
                    ],
                )
                pa_sb = pa_pool.tile([3, CHUNK], BF16)
                nc.sync.dma_start(
                    pa_sb[:], pa_sh[b][:, ck * CHUNK : (ck + 1) * CHUNK]
                )
                vn_sb = vn_pool.tile([128, VT_PER_CHUNK * H], BF16)
                nc.sync.dma_start(
                    vn_sb[:].rearrange("p (vt h) -> p vt h", h=H),
                    value_n[b].rearrange("(vt p) h -> p vt h", p=128)[
                        :, ck * VT_PER_CHUNK : (ck + 1) * VT_PER_CHUNK, :
                    ],
                )

                score_ps = sc_pool.tile([1, CHUNK], F32, tag="scps")
                t_tiles = []
                for dg in range(NDG):
                    e_ps = eps_pool.tile([128, CHUNK], F32)
                    nc.tensor.matmul(
                        e_ps[:],
                        w2_sb[:, dg * 128 : (dg + 1) * 128],
                        pa_sb[:],
                        start=True,
                        stop=False,
                    )
                    for ht in range(NHT):
                        nc.tensor.matmul(
                            e_ps[:],
                            wv_sb[:, ht * DIM + dg * 128 : ht * DIM + dg * 128 + 128],
                            vt_sb[:, ht * CHUNK : (ht + 1) * CHUNK],
                            start=False,
                            stop=(ht == NHT - 1),
                        )
                    t_sb = t_pool.tile([128, CHUNK], BF16)
                    nc.scalar.activation(
                        t_sb[:],
                        e_ps[:],
                        AF.Tanh,
                        bias=qpb_sb[:, b * NDG + dg : b * NDG + dg + 1],
                    )
                    t_tiles.append(t_sb)
                for dg in range(NDG):
                    nc.tensor.matmul(
                        score_ps[:],
                        wsc_sb[:, dg : dg + 1],
                        t_tiles[dg][:],
                        start=(dg == 0),
                        stop=(dg == NDG - 1),
                        skip_group_check=True,
                    )

                # sigmoid(score + b_score) -> s row chunk
                nc.scalar.activation(
                    s_row[:, ck * CHUNK : (ck + 1) * CHUNK],
                    score_ps[:],
                    AF.Sigmoid,
                    bias=bsc_sb[:],
                    accum_out=sparts[:, ck : ck + 1],
                )

                for vt in range(VT_PER_CHUNK):
                    vglob = ck * VT_PER_CHUNK + vt
                    # transpose s chunk-slice [1,128] -> column [128,1] via K=1 matmul
                    st_ps = sc_pool.tile([128, 1], F32, tag="scps", name="st_ps")
                    nc.tensor.matmul(
                        st_ps[:],
                        s_row[:, vglob * 128 : (vglob + 1) * 128],
                        one32[:],
                        start=True,
                        stop=True,
                    )
                    nc.vector.tensor_copy(s_col[:, vglob : vglob + 1], st_ps[:])
                    # context accumulation: U += s_col[vt] . value_n[vt]
                    for nh in range(2):
                        nc.tensor.matmul(
                            u_ps[:, nh * 512 : (nh + 1) * 512],
                            s_col[:, vglob : vglob + 1],
                            vn_sb[:, vt * H + nh * 512 : vt * H + (nh + 1) * 512],
                            start=(vglob == 0),
                            stop=(vglob == VLEN // 128 - 1),
                            skip_group_check=True,
                        )

            # S = sum(s); 1/S; scale attn row and context
            ssum_dummy = misc_pool.tile([1, NCHUNK], F32, tag="ssum_dummy")
            nc.scalar.activation(
                ssum_dummy[:],
                sparts[:],
                AF.Identity,
                accum_out=ssum[:],
            )
            recip = misc_pool.tile([1, 1], F32, tag="recip")
            nc.vector.reciprocal(recip[:], ssum[:])

            nc.vector.tensor_scalar_mul(s_row[:], s_row[:], recip[:])
            nc.sync.dma_start(attn_o[b], s_row[:])

            ctx_sb = misc_pool.tile([1, H], F32, tag="ctx")
            nc.vector.tensor_scalar_mul(ctx_sb[:], u_ps[:], recip[:])

            # transpose context row into combined^T columns (K=1 matmuls)
            for kt in range(NHT):
                ct_ps = sc_pool.tile([128, 1], F32, tag="scps", name="ct_ps")
                nc.tensor.matmul(
                    ct_ps[:],
                    ctx_sb[:, kt * 128 : (kt + 1) * 128],
                    one32[:],
                    start=True,
                    stop=True,
                )
                nc.vector.tensor_copy(
                    comb_sb[:, kt * BPC + b : kt * BPC + b + 1], ct_ps[:]
                )

        # ---- out_proj for all BPC batches: out[b, h] = comb^T.T @ w_outT ----
        op_ps = [
            sc_pool.tile([BPC, 512], F32, tag="scps", name=f"op_ps{i}")
            for i in range(2)
        ]
        for kt in range(NKT):
            wok = wo_pool.tile([128, H], BF16, name=f"wok{kt}", tag="wok")
            nc.sync.dma_start(wok[:], w_outT[kt * 128 : (kt + 1) * 128, :])
            for nh in range(2):
                nc.tensor.matmul(
                    op_ps[nh][:],
                    comb_sb[:, kt * BPC : (kt + 1) * BPC],
                    wok[:, nh * 512 : (nh + 1) * 512],
                    start=(kt == 0),
                    stop=(kt == NKT - 1),
                    skip_group_check=True,
                )
        for nh in range(2):
            nc.vector.tensor_tensor(
                out_sb[:, nh * 512 : (nh + 1) * 512],
                op_ps[nh][:],
                bo_sb[:, nh * 512 : (nh + 1) * 512],
                mybir.AluOpType.add,
            )
        nc.sync.dma_start(out_o[:], out_sb[:])

    nc.compile()
    return nc


def _get_nc():
    if "nc" not in _CACHE:
        _CACHE["nc"] = _build()
    return _CACHE["nc"]


def _host_prep(query, value, prev_attn, conv_w, conv_b, w_loc, w_q, w_v, bias,
               w_score, b_score, w_out, b_out):
    f32 = np.float32
    query = np.asarray(query, f32)        # (B, 1, H)
    value = np.asarray(value, f32)        # (B, V, H)
    prev_attn = np.asarray(prev_attn, f32)  # (B, V)
    conv_w = np.asarray(conv_w, f32)      # (C, 1, 3)
    conv_b = np.asarray(conv_b, f32)      # (C,)
    w_loc = np.asarray(w_loc, f32)        # (DIM, C)
    w_q = np.asarray(w_q, f32)            # (DIM, H)
    w_v = np.asarray(w_v, f32)            # (DIM, H)
    bias = np.asarray(bias, f32)          # (DIM,)
    w_score = np.asarray(w_score, f32)    # (1, DIM)
    b_score = np.asarray(b_score, f32)    # (1,)
    w_out = np.asarray(w_out, f32)        # (H, 2H)
    b_out = np.asarray(b_out, f32)        # (H,)

    qp = query[:, 0, :] @ w_q.T                       # (B, DIM)
    lb = w_loc @ conv_b                               # (DIM,)
    qpb_full = qp + bias[None, :] + lb[None, :]       # (B, DIM)
    W2 = np.einsum("ck,dc->kd", conv_w[:, 0, :], w_loc).astype(ml_dtypes.bfloat16)

    pa_sh_full = np.zeros((B, 3, VLEN), ml_dtypes.bfloat16)
    pa_sh_full[:, 0, 1:] = prev_attn[:, :-1]
    pa_sh_full[:, 1, :] = prev_attn
    pa_sh_full[:, 2, :-1] = prev_attn[:, 1:]

    value_t_full = np.ascontiguousarray(value.transpose(0, 2, 1).astype(ml_dtypes.bfloat16))

    w_vT_arr = np.ascontiguousarray(w_v.T.astype(ml_dtypes.bfloat16))
    w_outT_arr = np.ascontiguousarray(w_out.T.astype(ml_dtypes.bfloat16))
    w_sc_arr = np.ascontiguousarray(
        w_score[0].reshape(NDG, 128).T.astype(ml_dtypes.bfloat16)
    )
    b_sc_arr = b_score.reshape(1, 1).astype(f32)

    in_maps = []
    for c in range(NCORES):
        bs = slice(c * BPC, (c + 1) * BPC)
        qpb_c = qpb_full[bs]                          # (BPC, DIM)
        # qpb layout [128, b*NDG + dg] with qpb[p, ...] = qpb_c[b, dg*128+p]
        qpb_t = np.ascontiguousarray(
            qpb_c.reshape(BPC, NDG, 128).transpose(2, 0, 1).reshape(128, BPC * NDG)
        )
        # qT layout [128, ht*BPC + b] = query[b, 0, ht*128 + p]
        q_c = query[bs, 0, :]                         # (BPC, H)
        qT_arr = np.ascontiguousarray(
            q_c.reshape(BPC, NHT, 128)
            .transpose(2, 1, 0)
            .reshape(128, NHT * BPC)
            .astype(ml_dtypes.bfloat16)
        )
        in_maps.append({
            "value_t": np.ascontiguousarray(value_t_full[bs]),
            "value_n": np.ascontiguousarray(value[bs].astype(ml_dtypes.bfloat16)),
            "w_vT": w_vT_arr,
            "w2": W2,
            "pa_sh": np.ascontiguousarray(pa_sh_full[bs]),
            "qpb": qpb_t,
            "w_sc": w_sc_arr,
            "b_sc": b_sc_arr,
            "qT": qT_arr,
            "w_outT": w_outT_arr,
            "b_outr": np.ascontiguousarray(np.broadcast_to(b_out, (BPC, H))),
        })
    return in_maps


def _run(in_maps, trace=False):
    from concourse.bass_utils import run_bass_kernel_spmd

    nc = _get_nc()
    return run_bass_kernel_spmd(nc, in_maps, list(range(NCORES)), trace=trace)


def kernel(**inputs):
    in_maps = _host_prep(**inputs)
    res = _run(in_maps)
    out = np.concatenate(
        [res.results[c]["out_o"][:, None, :] for c in range(NCORES)], axis=0
    ).astype(np.float32)
    attn = np.concatenate(
        [res.results[c]["attn_o"] for c in range(NCORES)], axis=0
    ).astype(np.float32)
    return out, attn


# revision 15
# speedup vs baseline: 1.1331x; 1.1331x over previous
"""LocationAwareAttention Trainium2 kernel.

Full inputs in, full outputs out. Internally shards the batch dim (B=32)
across 8 NeuronCores (4 batches each) and runs a Bass/Tile kernel per core.

Math (per batch b):
    conv_feat = Conv1d(prev_attn, k=3, pad=1)           (V, C)
    e  = tanh(qp + value@w_v.T + conv_feat@w_loc.T + bias)   (V, DIM)
    score = e @ w_score + b_score                        (V,)
    s = sigmoid(score);  attn = s / sum(s)               (V,)
    context = attn @ value                               (H,)
    out = concat(context, query) @ w_out.T + b_out       (H,)

Device-side restructuring:
  - e computed transposed [DIM, V]: psum accumulation of
      (a) w_vT-tiles.T @ value_t-tiles     (K=h, 8 tiles)
      (b) W2.T @ shifted-prev-attn         (K=3; conv folded into W2)
    then ACT tanh with per-partition bias qpb = qp + bias + w_loc@conv_b.
  - score via M=1 matmuls (lhsT = w_score column per d-group).
  - s transposed to columns via K=1 matmuls against a ones scalar.
  - context accumulated unnormalized (U = sum_v s_v * value_v) during the
    same single pass over value (natural layout), scaled by 1/S at the end.
  - out_proj batched over the 4 local batches with M=4 (combined^T columns).
"""

import ml_dtypes
import numpy as np

B, VLEN, H, DIM, C = 32, 2048, 1024, 1024, 32
NCORES = 8
BPC = B // NCORES          # batches per core
CHUNK = 512                # v-chunk
NCHUNK = VLEN // CHUNK     # 4
VT_PER_CHUNK = CHUNK // 128  # 4 v-tiles of 128 per chunk
NDG = DIM // 128           # 8 d-groups
NHT = H // 128             # 8 h-tiles
NKT = 2 * H // 128         # 16 k-tiles for out_proj

_CACHE = {}


def _build():
    from contextlib import ExitStack

    import concourse.bacc as bacc
    import concourse.mybir as mybir
    import concourse.tile as tile

    F32R = mybir.dt.float32r
    F32 = mybir.dt.float32
    BF16 = mybir.dt.bfloat16
    AF = mybir.ActivationFunctionType

    nc = bacc.Bacc("TRN2", target_bir_lowering=False)

    value_t = nc.declare_dram_parameter("value_t", [BPC, H, VLEN], BF16, isOutput=False)
    value_n = nc.declare_dram_parameter("value_n", [BPC, VLEN, H], BF16, isOutput=False)
    w_vT = nc.declare_dram_parameter("w_vT", [H, DIM], BF16, isOutput=False)
    w2 = nc.declare_dram_parameter("w2", [3, DIM], BF16, isOutput=False)
    pa_sh = nc.declare_dram_parameter("pa_sh", [BPC, 3, VLEN], BF16, isOutput=False)
    qpb = nc.declare_dram_parameter("qpb", [128, BPC * NDG], F32, isOutput=False)
    w_sc = nc.declare_dram_parameter("w_sc", [128, NDG], BF16, isOutput=False)
    b_sc = nc.declare_dram_parameter("b_sc", [1, 1], F32, isOutput=False)
    qT = nc.declare_dram_parameter("qT", [128, NHT * BPC], BF16, isOutput=False)
    w_outT = nc.declare_dram_parameter("w_outT", [2 * H, H], BF16, isOutput=False)
    b_outr = nc.declare_dram_parameter("b_outr", [BPC, H], F32, isOutput=False)
    attn_o = nc.declare_dram_parameter("attn_o", [BPC, VLEN], F32, isOutput=True)
    out_o = nc.declare_dram_parameter("out_o", [BPC, H], F32, isOutput=True)

    with tile.TileContext(nc) as tc, ExitStack() as ctx:
        cpool = ctx.enter_context(tc.tile_pool(name="const", bufs=1))
        vt_pool = ctx.enter_context(tc.tile_pool(name="vt", bufs=3))
        vn_pool = ctx.enter_context(tc.tile_pool(name="vn", bufs=3))
        t_pool = ctx.enter_context(tc.tile_pool(name="t", bufs=9))
        srow_pool = ctx.enter_context(tc.tile_pool(name="srow", bufs=2))
        scol_pool = ctx.enter_context(tc.tile_pool(name="scol", bufs=2))
        misc_pool = ctx.enter_context(tc.tile_pool(name="misc", bufs=2))
        pa_pool = ctx.enter_context(tc.tile_pool(name="pa", bufs=2))
        wo_pool = ctx.enter_context(tc.tile_pool(name="wo", bufs=6))

        eps_pool = ctx.enter_context(tc.tile_pool(name="eps", bufs=2, space="PSUM"))
        sc_pool = ctx.enter_context(tc.tile_pool(name="scps", bufs=2, space="PSUM"))
        u_pool = ctx.enter_context(tc.tile_pool(name="ups", bufs=2, space="PSUM"))

        # ---- constants / weights (loaded once) ----
        wv_sb = cpool.tile([128, NHT * DIM], BF16, tag="wv")
        nc.sync.dma_start(
            wv_sb[:].rearrange("p (ht d) -> p ht d", d=DIM),
            w_vT.rearrange("(ht p) d -> p ht d", p=128),
        )
        w2_sb = cpool.tile([3, DIM], BF16, tag="w2")
        nc.sync.dma_start(w2_sb[:], w2[:])
        qpb_sb = cpool.tile([128, BPC * NDG], F32, tag="qpb")
        nc.sync.dma_start(qpb_sb[:], qpb[:])
        wsc_sb = cpool.tile([128, NDG], BF16, tag="wsc")
        nc.sync.dma_start(wsc_sb[:], w_sc[:])
        bsc_sb = cpool.tile([1, 1], F32, tag="bsc")
        nc.sync.dma_start(bsc_sb[:], b_sc[:])
        bo_sb = cpool.tile([BPC, H], F32, tag="bo")
        nc.sync.dma_start(bo_sb[:], b_outr[:])
        one32 = cpool.tile([1, 1], F32, tag="one32")
        nc.vector.memset(one32[:], 1.0)

        # combined^T columns [k-part, kt*BPC + b]; query half DMA'd directly
        comb_sb = cpool.tile([128, NKT * BPC], BF16, tag="comb")
        nc.sync.dma_start(comb_sb[:, NHT * BPC : NKT * BPC], qT[:])

        out_sb = cpool.tile([BPC, H], F32, tag="outsb")

        for b in range(BPC):
            s_row = srow_pool.tile([1, VLEN], F32)
            sparts = misc_pool.tile([1, NCHUNK], F32, tag="sparts")
            s_col = scol_pool.tile([128, VLEN // 128], BF16)
            u_ps = u_pool.tile([1, H], F32)
            ssum = misc_pool.tile([1, 1], F32, tag="ssum")

            for ck in range(NCHUNK):
                vt_sb = vt_pool.tile([128, NHT * CHUNK], BF16)
                nc.sync.dma_start(
                    vt_sb[:].rearrange("p (ht v) -> p ht v", v=CHUNK),
                    value_t[b].rearrange("(ht p) v -> p ht v", p=128)[
                        :, :, ck * CHUNK : (ck + 1) * CHUNK
                    ],
                )
                pa_sb = pa_pool.tile([3, CHUNK], BF16)
                nc.sync.dma_start(
                    pa_sb[:], pa_sh[b][:, ck * CHUNK : (ck + 1) * CHUNK]
                )
                vn_sb = vn_pool.tile([128, VT_PER_CHUNK * H], BF16)
                nc.sync.dma_start(
                    vn_sb[:].rearrange("p (vt h) -> p vt h", h=H),
                    value_n[b].rearrange("(vt p) h -> p vt h", p=128)[
                        :, ck * VT_PER_CHUNK : (ck + 1) * VT_PER_CHUNK, :
                    ],
                )

                score_ps = sc_pool.tile([1, CHUNK], F32, tag="scps")
                t_tiles = []
                for dg in range(NDG):
                    e_ps = eps_pool.tile([128, CHUNK], F32)
                    for ht in range(NHT):
                        nc.tensor.matmul(
                            e_ps[:],
                            wv_sb[:, ht * DIM + dg * 128 : ht * DIM + dg * 128 + 128],
                            vt_sb[:, ht * CHUNK : (ht + 1) * CHUNK],
                            start=(ht == 0),
                            stop=False,
                        )
                    nc.tensor.matmul(
                        e_ps[:],
                        w2_sb[:, dg * 128 : (dg + 1) * 128],
                        pa_sb[:],
                        start=False,
                        stop=True,
                    )
                    t_sb = t_pool.tile([128, CHUNK], BF16)
                    nc.scalar.activation(
                        t_sb[:],
                        e_ps[:],
                        AF.Tanh,
                        bias=qpb_sb[:, b * NDG + dg : b * NDG + dg + 1],
                    )
                    t_tiles.append(t_sb)
                    nc.tensor.matmul(
                        score_ps[:],
                        wsc_sb[:, dg : dg + 1],
                        t_sb[:],
                        start=(dg == 0),
                        stop=(dg == NDG - 1),
                        skip_group_check=True,
                    )

                # sigmoid(score + b_score) -> s row chunk
                nc.scalar.activation(
                    s_row[:, ck * CHUNK : (ck + 1) * CHUNK],
                    score_ps[:],
                    AF.Sigmoid,
                    bias=bsc_sb[:],
                    accum_out=sparts[:, ck : ck + 1],
                )

                for vt in range(VT_PER_CHUNK):
                    vglob = ck * VT_PER_CHUNK + vt
                    # transpose s chunk-slice [1,128] -> column [128,1] via K=1 matmul
                    st_ps = sc_pool.tile([128, 1], F32, tag="scps", name="st_ps")
                    nc.tensor.matmul(
                        st_ps[:],
                        s_row[:, vglob * 128 : (vglob + 1) * 128],
                        one32[:],
                        start=True,
                        stop=True,
                    )
                    nc.vector.tensor_copy(s_col[:, vglob : vglob + 1], st_ps[:])
                    # context accumulation: U += s_col[vt] . value_n[vt]
                    for nh in range(2):
                        nc.tensor.matmul(
                            u_ps[:, nh * 512 : (nh + 1) * 512],
                            s_col[:, vglob : vglob + 1],
                            vn_sb[:, vt * H + nh * 512 : vt * H + (nh + 1) * 512],
                            start=(vglob == 0),
                            stop=(vglob == VLEN // 128 - 1),
                            skip_group_check=True,
                        )

            # S = sum(s); 1/S; scale attn row and context
            ssum_dummy = misc_pool.tile([1, NCHUNK], F32, tag="ssum_dummy")
            nc.scalar.activation(
                ssum_dummy[:],
                sparts[:],
                AF.Identity,
                accum_out=ssum[:],
            )
            recip = misc_pool.tile([1, 1], F32, tag="recip")
            nc.vector.reciprocal(recip[:], ssum[:])

            nc.vector.tensor_scalar_mul(s_row[:], s_row[:], recip[:])
            nc.sync.dma_start(attn_o[b], s_row[:])

            ctx_sb = misc_pool.tile([1, H], F32, tag="ctx")
            nc.vector.tensor_scalar_mul(ctx_sb[:], u_ps[:], recip[:])

            # transpose context row into combined^T columns (K=1 matmuls)
            for kt in range(NHT):
                ct_ps = sc_pool.tile([128, 1], F32, tag="scps", name="ct_ps")
                nc.tensor.matmul(
                    ct_ps[:],
                    ctx_sb[:, kt * 128 : (kt + 1) * 128],
                    one32[:],
                    start=True,
                    stop=True,
                )
                nc.vector.tensor_copy(
                    comb_sb[:, kt * BPC + b : kt * BPC + b + 1], ct_ps[:]
                )

        # ---- out_proj for all BPC batches: out[b, h] = comb^T.T @ w_outT ----
        op_ps = [
            sc_pool.tile([BPC, 512], F32, tag="scps", name=f"op_ps{i}")
            for i in range(2)
        ]
        for kt in range(NKT):
            wok = wo_pool.tile([128, H], BF16, name=f"wok{kt}", tag="wok")
            nc.sync.dma_start(wok[:], w_outT[kt * 128 : (kt + 1) * 128, :])
            for nh in range(2):
                nc.tensor.matmul(
                    op_ps[nh][:],
                    comb_sb[:, kt * BPC : (kt + 1) * BPC],
                    wok[:, nh * 512 : (nh + 1) * 512],
                    start=(kt == 0),
                    stop=(kt == NKT - 1),
                    skip_group_check=True,
                )
        for nh in range(2):
            nc.vector.tensor_tensor(
                out_sb[:, nh * 512 : (nh + 1) * 512],
                op_ps[nh][:],
                bo_sb[:, nh * 512 : (nh + 1) * 512],
                mybir.AluOpType.add,
            )
        nc.sync.dma_start(out_o[:], out_sb[:])

    nc.compile()
    return nc


def _get_nc():
    if "nc" not in _CACHE:
        _CACHE["nc"] = _build()
    return _CACHE["nc"]


def _host_prep(query, value, prev_attn, conv_w, conv_b, w_loc, w_q, w_v, bias,
               w_score, b_score, w_out, b_out):
    f32 = np.float32
    query = np.asarray(query, f32)        # (B, 1, H)
    value = np.asarray(value, f32)        # (B, V, H)
    prev_attn = np.asarray(prev_attn, f32)  # (B, V)
    conv_w = np.asarray(conv_w, f32)      # (C, 1, 3)
    conv_b = np.asarray(conv_b, f32)      # (C,)
    w_loc = np.asarray(w_loc, f32)        # (DIM, C)
    w_q = np.asarray(w_q, f32)            # (DIM, H)
    w_v = np.asarray(w_v, f32)            # (DIM, H)
    bias = np.asarray(bias, f32)          # (DIM,)
    w_score = np.asarray(w_score, f32)    # (1, DIM)
    b_score = np.asarray(b_score, f32)    # (1,)
    w_out = np.asarray(w_out, f32)        # (H, 2H)
    b_out = np.asarray(b_out, f32)        # (H,)

    qp = query[:, 0, :] @ w_q.T                       # (B, DIM)
    lb = w_loc @ conv_b                               # (DIM,)
    qpb_full = qp + bias[None, :] + lb[None, :]       # (B, DIM)
    W2 = np.einsum("ck,dc->kd", conv_w[:, 0, :], w_loc).astype(ml_dtypes.bfloat16)

    pa_sh_full = np.zeros((B, 3, VLEN), ml_dtypes.bfloat16)
    pa_sh_full[:, 0, 1:] = prev_attn[:, :-1]
    pa_sh_full[:, 1, :] = prev_attn
    pa_sh_full[:, 2, :-1] = prev_attn[:, 1:]

    value_t_full = np.ascontiguousarray(value.transpose(0, 2, 1).astype(ml_dtypes.bfloat16))

    w_vT_arr = np.ascontiguousarray(w_v.T.astype(ml_dtypes.bfloat16))
    w_outT_arr = np.ascontiguousarray(w_out.T.astype(ml_dtypes.bfloat16))
    w_sc_arr = np.ascontiguousarray(
        w_score[0].reshape(NDG, 128).T.astype(ml_dtypes.bfloat16)
    )
    b_sc_arr = b_score.reshape(1, 1).astype(f32)

    in_maps = []
    for c in range(NCORES):
        bs = slice(c * BPC, (c + 1) * BPC)
        qpb_c = qpb_full[bs]                          # (BPC, DIM)
        # qpb layout [128, b*NDG + dg] with qpb[p, ...] = qpb_c[b, dg*128+p]
        qpb_t = np.ascontiguousarray(
            qpb_c.reshape(BPC, NDG, 128).transpose(2, 0, 1).reshape(128, BPC * NDG)
        )
        # qT layout [128, ht*BPC + b] = query[b, 0, ht*128 + p]
        q_c = query[bs, 0, :]                         # (BPC, H)
        qT_arr = np.ascontiguousarray(
            q_c.reshape(BPC, NHT, 128)
            .transpose(2, 1, 0)
            .reshape(128, NHT * BPC)
            .astype(ml_dtypes.bfloat16)
        )
        in_maps.append({
            "value_t": np.ascontiguousarray(value_t_full[bs]),
            "value_n": np.ascontiguousarray(value[bs].astype(ml_dtypes.bfloat16)),
            "w_vT": w_vT_arr,
            "w2": W2,
            "pa_sh": np.ascontiguousarray(pa_sh_full[bs]),
            "qpb": qpb_t,
            "w_sc": w_sc_arr,
            "b_sc": b_sc_arr,
            "qT": qT_arr,
            "w_outT": w_outT_arr,
            "b_outr": np.ascontiguousarray(np.broadcast_to(b_out, (BPC, H))),
        })
    return in_maps


def _run(in_maps, trace=False):
    from concourse.bass_utils import run_bass_kernel_spmd

    nc = _get_nc()
    return run_bass_kernel_spmd(nc, in_maps, list(range(NCORES)), trace=trace)


def kernel(**inputs):
    in_maps = _host_prep(**inputs)
    res = _run(in_maps)
    out = np.concatenate(
        [res.results[c]["out_o"][:, None, :] for c in range(NCORES)], axis=0
    ).astype(np.float32)
    attn = np.concatenate(
        [res.results[c]["attn_o"] for c in range(NCORES)], axis=0
    ).astype(np.float32)
    return out, attn
